# revision 26
# baseline (speedup 1.0000x reference)
"""Trainium2 Bass kernel for the LELoss problem (raw Bass, 8-core SPMD).

loss = mean_b ||x_b - dec_b||^2
     + 1.1 * mean_b ||enc_b - (lat @ rsrA.T)_b||^2
     + 0.1 * mean((rsrA.T @ rsrA - I)^2)

(The knn/cdist/topk in the original module is dead code - its result is never
used - so the returned loss reduces to the three terms above.)

Per-core algebra (batch shard of R=1024 rows):
  sum||enc - lat@A.T||^2 = sum(enc^2) - 2*sum(M .* A) + sum(L .* G0)
      with M = enc.T @ lat [E,I], L = lat.T @ lat [I,I], G0 = A.T @ A [I,I]
  sum((G0 - I)^2) = sum(G0^2) - 2*sum(A^2) + I_dim
All partial sums land in columns of a [128,18] SBUF accumulator S which is
DMA'd out per core; the host collapses partitions/cores and applies weights.

DMA strategy (v7): per-queue HWDGE throughput is ~max(20ns, bytes/350GB/s)
per descriptor and the engine bus caps at ~435 GB/s, so 8KB descriptors let
two queues exceed the 2x206 GB/s all-4KB rate -- but only in unmixed phases
(mixed 8KB/4KB phases measured sags, and a big first item delays the other
queue's start by its generation time).  Layout:
  - each queue STARTS with one 4KB half-combo (fast generator handoff),
  - then an all-8KB phase: tiles 0..3 as row-interleaved [x|dec] combos
    ([128,2048], 8KB rows, self-contained, one queue each),
  - then an all-4KB phase: tiles 4..7 as column-half combos [x[:,c:c+512] |
    dec[:,c:c+512]] ([128,1024], 4KB rows) staggered across both queue
    tails, so the end-of-stream compute works on 512-column pieces.
In-place everywhere: sub writes the x half, square dst is the dec half.
The program-last half (b6) gets its square on DVE (stt, cheap accumulator
read); the other halves square on ACT.
"""

import contextlib

import numpy as np

try:
    import concourse.bass as bass
except ImportError:  # pragma: no cover - grading env fallback
    import sys

    sys.path.insert(0, "/opt/trn_rl_repo")
    import concourse.bass as bass

from concourse import mybir
from concourse.bass_utils import run_bass_kernel_spmd

N_CORES = 8
B, D, E, I = 8192, 1024, 128, 20
R = B // N_CORES  # rows per core = 1024
P = 128  # SBUF partitions
RT = R // P  # row tiles per core = 8
NC_ = 4  # full [x|dec] combo tiles (tiles 0..3)
NH = 8  # column-half combos (tiles 4..7)
DH = D // 2  # 512
S_COLS = 18
F32 = mybir.dt.float32

ENC_W = RT * E  # 1024 cols of packed enc
LAT_W = RT * I  # 160 cols of packed lat
PACK_W = ENC_W + LAT_W + I  # 1204

# S column map
C_TILE0 = 0  # 0..3 combo (x-dec)^2 sums (ACT)
C_HA = 4  # ACT half squares in production order b0,b1,b2,b3,b4,b5,b7 -> 4..10
C_HV = 11  # program-last half (b6) square (DVE stt)
C_ENC = 12  # sum(enc^2) (ACT)
C_A2 = 13  # sum(A^2) (ACT)
C_CROSS = 14  # sum(M .* A) (DVE)
C_ZSQ = 15  # sum(L .* G0) (DVE)
C_G2 = 16  # sum(G0^2) (DVE)

TRACE = False
LAST_RESULT = None
WAIT_OUT = True

_NC = None


def _build_nc():
    nc = bass.Bass()
    xd = nc.dram_tensor("xd", [NC_ * P, 2 * D], F32, kind="ExternalInput")
    th = nc.dram_tensor("th", [NH * P, D], F32, kind="ExternalInput")
    pack = nc.dram_tensor("pack", [P, PACK_W], F32, kind="ExternalInput")
    out = nc.dram_tensor("out", [P, S_COLS], F32, kind="ExternalOutput")

    Square = mybir.ActivationFunctionType.Square
    mult = mybir.AluOpType.mult
    bypass = mybir.AluOpType.bypass

    ctx = contextlib.ExitStack()
    with ctx:
        cb = [
            ctx.enter_context(nc.sbuf_tensor(f"cb{t}", [P, 2 * D], F32))
            for t in range(NC_)
        ]
        hb = [
            ctx.enter_context(nc.sbuf_tensor(f"hb{j}", [P, D], F32))
            for j in range(NH)
        ]
        pk_sb = ctx.enter_context(nc.sbuf_tensor("pk_sb", [P, PACK_W], F32))
        S = ctx.enter_context(nc.sbuf_tensor("S", [P, S_COLS], F32))
        G_sb = ctx.enter_context(nc.sbuf_tensor("G_sb", [I, I], F32))
        scr_m = ctx.enter_context(nc.sbuf_tensor("scr_m", [E, I], F32))
        scr_i = ctx.enter_context(nc.sbuf_tensor("scr_i", [I, I], F32))
        scr_a = ctx.enter_context(nc.sbuf_tensor("scr_a", [E, I], F32))

        psum_M = ctx.enter_context(nc.psum_tensor([E, I], F32))
        psum_L = ctx.enter_context(nc.psum_tensor([I, I], F32))
        psum_G = ctx.enter_context(nc.psum_tensor([I, I], F32))

        # per-queue cumulative completion sems: HWDGE queue completions
        # retire in order, so item k's data is resident once the queue's
        # sem reaches 16*(k+1)
        s_q1 = ctx.enter_context(nc.semaphore("s_q1"))
        s_q2 = ctx.enter_context(nc.semaphore("s_q2"))
        s_init = ctx.enter_context(nc.semaphore("s_init"))
        s_sub = ctx.enter_context(nc.semaphore("s_sub"))
        s_pe = ctx.enter_context(nc.semaphore("s_pe"))
        s_sq = ctx.enter_context(nc.semaphore("s_sq"))
        s_vfin = ctx.enter_context(nc.semaphore("s_vfin"))
        s_out = ctx.enter_context(nc.semaphore("s_out"))

        block = ctx.enter_context(nc.Block())

        def enc_t(t):
            return pk_sb[:, t * E : (t + 1) * E]

        def lat_t(t):
            return pk_sb[:, ENC_W + t * I : ENC_W + (t + 1) * I]

        a_sb = pk_sb[:, ENC_W + LAT_W : PACK_W]

        def dma_h(eng, j, s_q):
            eng.dma_start(out=hb[j][:, :], in_=th[j * P : (j + 1) * P, :]).then_inc(
                s_q, 16
            )

        def dma_c(eng, t, s_q):
            eng.dma_start(out=cb[t][:, :], in_=xd[t * P : (t + 1) * P, :]).then_inc(
                s_q, 16
            )

        # queue item positions (1-based) -> cumulative sem thresholds
        Q1_B0, Q1_C0, Q1_PK, Q1_C2, Q1_B2, Q1_B4, Q1_B6 = (
            16, 32, 48, 64, 80, 96, 112,
        )
        Q2_B1, Q2_C1, Q2_C3, Q2_B3, Q2_B5, Q2_B7 = 16, 32, 48, 64, 80, 96

        @block.sync
        def _(sync):
            # SP queue (~4.81MB): b0, c0, pack, c2, b2, b4, b6
            dma_h(sync, 0, s_q1)
            dma_c(sync, 0, s_q1)
            sync.dma_start(out=pk_sb[:, :], in_=pack[:, :]).then_inc(s_q1, 16)
            dma_c(sync, 2, s_q1)
            dma_h(sync, 2, s_q1)
            dma_h(sync, 4, s_q1)
            dma_h(sync, 6, s_q1)
            # ship the accumulator once every column is final
            sync.wait_ge(s_sq, 13)
            sync.wait_ge(s_vfin, 2)
            sync.dma_start(out=out[:, :], in_=S[:, :]).then_inc(s_out, 16)
            if WAIT_OUT:
                sync.wait_ge(s_out, 16)

        @block.scalar
        def _(scalar):
            # ACT queue (~4.19MB): b1, c1, c3, b3, b5, b7
            dma_h(scalar, 1, s_q2)
            dma_c(scalar, 1, s_q2)
            dma_c(scalar, 3, s_q2)
            dma_h(scalar, 3, s_q2)
            dma_h(scalar, 5, s_q2)
            dma_h(scalar, 7, s_q2)
            # squares, in DVE production (s_sub) order:
            # b0(1), b1(2), c0(3), c1(4), c2(5), c3(6), b2(7), b3(8),
            # b4(9), b5(10), b7(11); b6 is squared on DVE.
            scalar.wait_ge(s_init, 1)
            for k, j in enumerate((0, 1)):
                scalar.wait_ge(s_sub, k + 1)
                nc.scalar.activation(
                    out=hb[j][:, DH:], in_=hb[j][:, 0:DH], func=Square,
                    accum_out=S[:, C_HA + k : C_HA + k + 1],
                ).then_inc(s_sq, 1)
            for t in range(NC_):
                scalar.wait_ge(s_sub, t + 3)
                nc.scalar.activation(
                    out=cb[t][:, D:], in_=cb[t][:, 0:D], func=Square,
                    accum_out=S[:, t : t + 1],
                ).then_inc(s_sq, 1)
                if t == 1:
                    scalar.wait_ge(s_q1, Q1_PK)
                    nc.scalar.activation(
                        out=cb[0][:, D:], in_=pk_sb[:, 0:ENC_W], func=Square,
                        accum_out=S[:, C_ENC : C_ENC + 1],
                    ).then_inc(s_sq, 1)
                    nc.scalar.activation(
                        out=scr_a[:, :], in_=a_sb, func=Square,
                        accum_out=S[:E, C_A2 : C_A2 + 1],
                    ).then_inc(s_sq, 1)
            for k, j in enumerate((2, 3, 4, 5, 7)):
                scalar.wait_ge(s_sub, k + 7)
                nc.scalar.activation(
                    out=hb[j][:, DH:], in_=hb[j][:, 0:DH], func=Square,
                    accum_out=S[:, C_HA + 2 + k : C_HA + 3 + k],
                ).then_inc(s_sq, 1)

        @block.vector
        def _(vector):
            nc.vector.memset(S[:, :], 0.0).then_inc(s_init, 1)

            def sub_h(j, s_q, thr):
                vector.wait_ge(s_q, thr)
                nc.vector.tensor_sub(
                    hb[j][:, 0:DH], hb[j][:, 0:DH], hb[j][:, DH:]
                ).then_inc(s_sub, 1)

            def sub_c(t, s_q, thr):
                vector.wait_ge(s_q, thr)
                nc.vector.tensor_sub(
                    cb[t][:, 0:D], cb[t][:, 0:D], cb[t][:, D:]
                ).then_inc(s_sub, 1)

            # starters
            sub_h(0, s_q1, Q1_B0)
            sub_h(1, s_q2, Q2_B1)
            # combo phase
            sub_c(0, s_q1, Q1_C0)
            sub_c(1, s_q2, Q2_C1)
            sub_c(2, s_q1, Q1_C2)
            sub_c(3, s_q2, Q2_C3)
            # tiny fused reductions over the PCA/proj matmul results, in the
            # slack before the tail halves arrive
            vector.wait_ge(s_pe, 1)
            nc.vector.tensor_copy(G_sb[:, :], psum_G[:, :])
            nc.vector.scalar_tensor_tensor(
                out=scr_m[:, :], in0=psum_M[:, :], scalar=1.0, in1=a_sb,
                op0=bypass, op1=mult, accum_out=S[:E, C_CROSS : C_CROSS + 1],
            )
            nc.vector.scalar_tensor_tensor(
                out=scr_i[:, :], in0=psum_L[:, :], scalar=1.0, in1=G_sb[:, :],
                op0=bypass, op1=mult, accum_out=S[:I, C_ZSQ : C_ZSQ + 1],
            )
            nc.vector.scalar_tensor_tensor(
                out=scr_i[:, :], in0=G_sb[:, :], scalar=1.0, in1=G_sb[:, :],
                op0=bypass, op1=mult, accum_out=S[:I, C_G2 : C_G2 + 1],
            ).then_inc(s_vfin, 1)
            # tail halves; b6 (SP queue end) is program-last, squared here
            sub_h(2, s_q1, Q1_B2)
            sub_h(3, s_q2, Q2_B3)
            sub_h(4, s_q1, Q1_B4)
            sub_h(5, s_q2, Q2_B5)
            sub_h(7, s_q2, Q2_B7)
            sub_h(6, s_q1, Q1_B6)
            nc.vector.scalar_tensor_tensor(
                out=hb[6][:, DH:], in0=hb[6][:, 0:DH], scalar=1.0,
                in1=hb[6][:, 0:DH], op0=bypass, op1=mult,
                accum_out=S[:, C_HV : C_HV + 1],
            ).then_inc(s_vfin, 1)

        @block.tensor
        def _(tensor):
            tensor.wait_ge(s_q1, Q1_PK)
            for t in range(RT):
                nc.tensor.matmul(
                    psum_M[:, :], lhsT=enc_t(t), rhs=lat_t(t),
                    start=(t == 0), stop=(t == RT - 1),
                )
            for t in range(RT):
                nc.tensor.matmul(
                    psum_L[:, :], lhsT=lat_t(t), rhs=lat_t(t),
                    start=(t == 0), stop=(t == RT - 1),
                )
            nc.tensor.matmul(
                psum_G[:, :], lhsT=a_sb, rhs=a_sb, start=True, stop=True
            ).then_inc(s_pe, 1)

    return nc


def kernel(x, encoded, latent, decoded, rsrA):
    global _NC, LAST_RESULT
    if _NC is None:
        _NC = _build_nc()

    x = np.ascontiguousarray(x, dtype=np.float32)
    decoded = np.ascontiguousarray(decoded, dtype=np.float32)
    encoded = np.ascontiguousarray(encoded, dtype=np.float32)
    latent = np.ascontiguousarray(latent, dtype=np.float32)
    rsrA = np.ascontiguousarray(rsrA, dtype=np.float32)

    in_maps = []
    for c in range(N_CORES):
        sl = slice(c * R, (c + 1) * R)
        encr = encoded[sl].reshape(P, ENC_W)
        latr = latent[sl].reshape(P, LAT_W)
        pk = np.concatenate([encr, latr, rsrA], axis=1)
        xs, ds = x[sl], decoded[sl]
        # combos: [x_row | dec_row] for tiles 0..3
        xd = np.empty((NC_ * P, 2 * D), dtype=np.float32)
        xd[:, :D] = xs[: NC_ * P]
        xd[:, D:] = ds[: NC_ * P]
        # half-combos [x[:, c:c+512] | dec[:, c:c+512]] for tiles 4..7;
        # block j = 2*(t-4) + (0 if c0==0 else 1)
        th = np.empty((NH * P, D), dtype=np.float32)
        for t in range(4, 8):
            rs = slice(t * P, (t + 1) * P)
            for half, c0 in enumerate((0, DH)):
                j = 2 * (t - 4) + half
                th[j * P : (j + 1) * P, :DH] = xs[rs, c0 : c0 + DH]
                th[j * P : (j + 1) * P, DH:] = ds[rs, c0 : c0 + DH]
        in_maps.append(
            {"xd": xd, "th": th, "pack": np.ascontiguousarray(pk)}
        )

    res = run_bass_kernel_spmd(_NC, in_maps, core_ids=list(range(N_CORES)), trace=TRACE)
    LAST_RESULT = res

    o = np.stack([r["out"] for r in res.results]).astype(np.float64)  # [8,128,18]
    cols = o.sum(axis=(0, 1))  # [18]
    s_recon = cols[0 : C_HV + 1].sum()  # combos 0..3 + all 8 halves
    s_enc2 = cols[C_ENC]
    s_cross = cols[C_CROSS]
    s_zsq = cols[C_ZSQ]
    g2 = o[0, :, C_G2].sum()  # replicated terms: core 0 only
    ra2 = o[0, :, C_A2].sum()

    pca_sq = s_enc2 - 2.0 * s_cross + s_zsq
    proj_sq = g2 - 2.0 * ra2 + float(I)
    loss = s_recon / B + 1.1 * pca_sq / B + 0.1 * proj_sq / (I * I)
    return np.asarray(loss, dtype=np.float32)


# revision 27
# speedup vs baseline: 1.0959x; 1.0959x over previous
"""Trainium2 Bass kernel for the LELoss problem (raw Bass, 8-core SPMD).

loss = mean_b ||x_b - dec_b||^2
     + 1.1 * mean_b ||enc_b - (lat @ rsrA.T)_b||^2
     + 0.1 * mean((rsrA.T @ rsrA - I)^2)

(The knn/cdist/topk in the original module is dead code - its result is never
used - so the returned loss reduces to the three terms above.)

Per-core algebra (batch shard of R=1024 rows):
  sum||enc - lat@A.T||^2 = sum(enc^2) - 2*sum(M .* A) + sum(L .* G0)
      with M = enc.T @ lat [E,I], L = lat.T @ lat [I,I], G0 = A.T @ A [I,I]
  sum((G0 - I)^2) = sum(G0^2) - 2*sum(A^2) + I_dim
All partial sums land in columns of a [128,18] SBUF accumulator S which is
DMA'd out per core; the host collapses partitions/cores and applies weights.

DMA strategy (v7): per-queue HWDGE throughput is ~max(20ns, bytes/350GB/s)
per descriptor and the engine bus caps at ~435 GB/s, so 8KB descriptors let
two queues exceed the 2x206 GB/s all-4KB rate -- but only in unmixed phases
(mixed 8KB/4KB phases measured sags, and a big first item delays the other
queue's start by its generation time).  Layout:
  - each queue STARTS with one 4KB half-combo (fast generator handoff),
  - then an all-8KB phase: tiles 0..3 as row-interleaved [x|dec] combos
    ([128,2048], 8KB rows, self-contained, one queue each),
  - then an all-4KB phase: tiles 4..7 as column-half combos [x[:,c:c+512] |
    dec[:,c:c+512]] ([128,1024], 4KB rows) staggered across both queue
    tails, so the end-of-stream compute works on 512-column pieces.
In-place everywhere: sub writes the x half, square dst is the dec half.
The program-last half (b6) gets its square on DVE (stt, cheap accumulator
read); the other halves square on ACT.
"""

import contextlib

import numpy as np

try:
    import concourse.bass as bass
except ImportError:  # pragma: no cover - grading env fallback
    import sys

    sys.path.insert(0, "/opt/trn_rl_repo")
    import concourse.bass as bass

from concourse import mybir
from concourse.bass_utils import run_bass_kernel_spmd

N_CORES = 8
B, D, E, I = 8192, 1024, 128, 20
R = B // N_CORES  # rows per core = 1024
P = 128  # SBUF partitions
RT = R // P  # row tiles per core = 8
NC_ = 4  # full [x|dec] combo tiles (tiles 0..3)
NH = 8  # column-half combos (tiles 4..7)
DH = D // 2  # 512
S_COLS = 18
F32 = mybir.dt.float32

ENC_W = RT * E  # 1024 cols of packed enc
LAT_W = RT * I  # 160 cols of packed lat
PACK_W = ENC_W + LAT_W + I  # 1204

# S column map
C_TILE0 = 0  # 0..3 combo (x-dec)^2 sums (ACT)
C_HA = 4  # ACT half squares in production order b0,b1,b2,b3,b4,b5,b7 -> 4..10
C_HV = 11  # program-last half (b6) square (DVE stt)
C_ENC = 12  # sum(enc^2) (ACT)
C_A2 = 13  # sum(A^2) (ACT)
C_CROSS = 14  # sum(M .* A) (DVE)
C_ZSQ = 15  # sum(L .* G0) (DVE)
C_G2 = 16  # sum(G0^2) (DVE)

TRACE = False
LAST_RESULT = None
WAIT_OUT = True

_NC = None


def _build_nc():
    nc = bass.Bass()
    xd = nc.dram_tensor("xd", [NC_ * P, 2 * D], F32, kind="ExternalInput")
    th = nc.dram_tensor("th", [NH * P, D], F32, kind="ExternalInput")
    pack = nc.dram_tensor("pack", [P, PACK_W], F32, kind="ExternalInput")
    out = nc.dram_tensor("out", [P, S_COLS], F32, kind="ExternalOutput")

    Square = mybir.ActivationFunctionType.Square
    mult = mybir.AluOpType.mult
    bypass = mybir.AluOpType.bypass

    ctx = contextlib.ExitStack()
    with ctx:
        cb = [
            ctx.enter_context(nc.sbuf_tensor(f"cb{t}", [P, 2 * D], F32))
            for t in range(NC_)
        ]
        hb = [
            ctx.enter_context(nc.sbuf_tensor(f"hb{j}", [P, D], F32))
            for j in range(NH)
        ]
        pk_sb = ctx.enter_context(nc.sbuf_tensor("pk_sb", [P, PACK_W], F32))
        S = ctx.enter_context(nc.sbuf_tensor("S", [P, S_COLS], F32))
        G_sb = ctx.enter_context(nc.sbuf_tensor("G_sb", [I, I], F32))
        scr_m = ctx.enter_context(nc.sbuf_tensor("scr_m", [E, I], F32))
        scr_i = ctx.enter_context(nc.sbuf_tensor("scr_i", [I, I], F32))
        scr_a = ctx.enter_context(nc.sbuf_tensor("scr_a", [E, I], F32))

        psum_M = ctx.enter_context(nc.psum_tensor([E, I], F32))
        psum_L = ctx.enter_context(nc.psum_tensor([I, I], F32))
        psum_G = ctx.enter_context(nc.psum_tensor([I, I], F32))

        s_c = [ctx.enter_context(nc.semaphore(f"s_c{t}")) for t in range(NC_)]
        s_h = [ctx.enter_context(nc.semaphore(f"s_h{j}")) for j in range(NH)]
        s_small = ctx.enter_context(nc.semaphore("s_small"))
        s_init = ctx.enter_context(nc.semaphore("s_init"))
        s_sub = ctx.enter_context(nc.semaphore("s_sub"))
        s_pe = ctx.enter_context(nc.semaphore("s_pe"))
        s_sq = ctx.enter_context(nc.semaphore("s_sq"))
        s_vfin = ctx.enter_context(nc.semaphore("s_vfin"))
        s_out = ctx.enter_context(nc.semaphore("s_out"))

        block = ctx.enter_context(nc.Block())

        def enc_t(t):
            return pk_sb[:, t * E : (t + 1) * E]

        def lat_t(t):
            return pk_sb[:, ENC_W + t * I : ENC_W + (t + 1) * I]

        a_sb = pk_sb[:, ENC_W + LAT_W : PACK_W]

        def dma_h(eng, j):
            eng.dma_start(out=hb[j][:, :], in_=th[j * P : (j + 1) * P, :]).then_inc(
                s_h[j], 16
            )

        def dma_c(eng, t):
            eng.dma_start(out=cb[t][:, :], in_=xd[t * P : (t + 1) * P, :]).then_inc(
                s_c[t], 16
            )

        @block.sync
        def _(sync):
            # SP queue (~4.81MB): b0, c0, pack, c2, b2, b4, b6
            dma_h(sync, 0)
            dma_c(sync, 0)
            sync.dma_start(out=pk_sb[:, :], in_=pack[:, :]).then_inc(s_small, 16)
            dma_c(sync, 2)
            dma_h(sync, 2)
            dma_h(sync, 4)
            dma_h(sync, 6)
            # ship the accumulator once every column is final
            sync.wait_ge(s_sq, 13)
            sync.wait_ge(s_vfin, 2)
            sync.dma_start(out=out[:, :], in_=S[:, :]).then_inc(s_out, 16)
            if WAIT_OUT:
                sync.wait_ge(s_out, 16)

        @block.scalar
        def _(scalar):
            # ACT queue (~4.19MB): b1, c1, c3, b3, b5, b7
            dma_h(scalar, 1)
            dma_c(scalar, 1)
            dma_c(scalar, 3)
            dma_h(scalar, 3)
            dma_h(scalar, 5)
            dma_h(scalar, 7)
            # squares, in DVE production (s_sub) order:
            # b0(1), b1(2), c0(3), c1(4), c2(5), c3(6), b2(7), b3(8),
            # b4(9), b5(10), b7(11); b6 is squared on DVE.
            scalar.wait_ge(s_init, 1)
            for k, j in enumerate((0, 1)):
                scalar.wait_ge(s_sub, k + 1)
                nc.scalar.activation(
                    out=hb[j][:, DH:], in_=hb[j][:, 0:DH], func=Square,
                    accum_out=S[:, C_HA + k : C_HA + k + 1],
                ).then_inc(s_sq, 1)
            for t in range(NC_):
                scalar.wait_ge(s_sub, t + 3)
                nc.scalar.activation(
                    out=cb[t][:, D:], in_=cb[t][:, 0:D], func=Square,
                    accum_out=S[:, t : t + 1],
                ).then_inc(s_sq, 1)
                if t == 1:
                    scalar.wait_ge(s_small, 16)
                    nc.scalar.activation(
                        out=cb[0][:, D:], in_=pk_sb[:, 0:ENC_W], func=Square,
                        accum_out=S[:, C_ENC : C_ENC + 1],
                    ).then_inc(s_sq, 1)
                    nc.scalar.activation(
                        out=scr_a[:, :], in_=a_sb, func=Square,
                        accum_out=S[:E, C_A2 : C_A2 + 1],
                    ).then_inc(s_sq, 1)
            for k, j in enumerate((2, 3, 4, 5, 7)):
                scalar.wait_ge(s_sub, k + 7)
                nc.scalar.activation(
                    out=hb[j][:, DH:], in_=hb[j][:, 0:DH], func=Square,
                    accum_out=S[:, C_HA + 2 + k : C_HA + 3 + k],
                ).then_inc(s_sq, 1)

        @block.vector
        def _(vector):
            nc.vector.memset(S[:, :], 0.0).then_inc(s_init, 1)

            def sub_h(j):
                vector.wait_ge(s_h[j], 16)
                nc.vector.tensor_sub(
                    hb[j][:, 0:DH], hb[j][:, 0:DH], hb[j][:, DH:]
                ).then_inc(s_sub, 1)

            # starters
            sub_h(0)
            sub_h(1)
            # combo phase
            for t in range(NC_):
                vector.wait_ge(s_c[t], 16)
                nc.vector.tensor_sub(
                    cb[t][:, 0:D], cb[t][:, 0:D], cb[t][:, D:]
                ).then_inc(s_sub, 1)
            # tiny fused reductions over the PCA/proj matmul results, in the
            # slack before the tail halves arrive
            vector.wait_ge(s_pe, 1)
            nc.vector.tensor_copy(G_sb[:, :], psum_G[:, :])
            nc.vector.scalar_tensor_tensor(
                out=scr_m[:, :], in0=psum_M[:, :], scalar=1.0, in1=a_sb,
                op0=bypass, op1=mult, accum_out=S[:E, C_CROSS : C_CROSS + 1],
            )
            nc.vector.scalar_tensor_tensor(
                out=scr_i[:, :], in0=psum_L[:, :], scalar=1.0, in1=G_sb[:, :],
                op0=bypass, op1=mult, accum_out=S[:I, C_ZSQ : C_ZSQ + 1],
            )
            nc.vector.scalar_tensor_tensor(
                out=scr_i[:, :], in0=G_sb[:, :], scalar=1.0, in1=G_sb[:, :],
                op0=bypass, op1=mult, accum_out=S[:I, C_G2 : C_G2 + 1],
            ).then_inc(s_vfin, 1)
            # tail halves; b6 (SP queue end) is program-last, squared here
            for j in (2, 3, 4, 5, 7, 6):
                sub_h(j)
            nc.vector.scalar_tensor_tensor(
                out=hb[6][:, DH:], in0=hb[6][:, 0:DH], scalar=1.0,
                in1=hb[6][:, 0:DH], op0=bypass, op1=mult,
                accum_out=S[:, C_HV : C_HV + 1],
            ).then_inc(s_vfin, 1)

        @block.tensor
        def _(tensor):
            tensor.wait_ge(s_small, 16)
            for t in range(RT):
                nc.tensor.matmul(
                    psum_M[:, :], lhsT=enc_t(t), rhs=lat_t(t),
                    start=(t == 0), stop=(t == RT - 1),
                )
            for t in range(RT):
                nc.tensor.matmul(
                    psum_L[:, :], lhsT=lat_t(t), rhs=lat_t(t),
                    start=(t == 0), stop=(t == RT - 1),
                )
            nc.tensor.matmul(
                psum_G[:, :], lhsT=a_sb, rhs=a_sb, start=True, stop=True
            ).then_inc(s_pe, 1)

    return nc


def kernel(x, encoded, latent, decoded, rsrA):
    global _NC, LAST_RESULT
    if _NC is None:
        _NC = _build_nc()

    x = np.ascontiguousarray(x, dtype=np.float32)
    decoded = np.ascontiguousarray(decoded, dtype=np.float32)
    encoded = np.ascontiguousarray(encoded, dtype=np.float32)
    latent = np.ascontiguousarray(latent, dtype=np.float32)
    rsrA = np.ascontiguousarray(rsrA, dtype=np.float32)

    in_maps = []
    for c in range(N_CORES):
        sl = slice(c * R, (c + 1) * R)
        encr = encoded[sl].reshape(P, ENC_W)
        latr = latent[sl].reshape(P, LAT_W)
        pk = np.concatenate([encr, latr, rsrA], axis=1)
        xs, ds = x[sl], decoded[sl]
        # combos: [x_row | dec_row] for tiles 0..3
        xd = np.empty((NC_ * P, 2 * D), dtype=np.float32)
        xd[:, :D] = xs[: NC_ * P]
        xd[:, D:] = ds[: NC_ * P]
        # half-combos [x[:, c:c+512] | dec[:, c:c+512]] for tiles 4..7;
        # block j = 2*(t-4) + (0 if c0==0 else 1)
        th = np.empty((NH * P, D), dtype=np.float32)
        for t in range(4, 8):
            rs = slice(t * P, (t + 1) * P)
            for half, c0 in enumerate((0, DH)):
                j = 2 * (t - 4) + half
                th[j * P : (j + 1) * P, :DH] = xs[rs, c0 : c0 + DH]
                th[j * P : (j + 1) * P, DH:] = ds[rs, c0 : c0 + DH]
        in_maps.append(
            {"xd": xd, "th": th, "pack": np.ascontiguousarray(pk)}
        )

    res = run_bass_kernel_spmd(_NC, in_maps, core_ids=list(range(N_CORES)), trace=TRACE)
    LAST_RESULT = res

    o = np.stack([r["out"] for r in res.results]).astype(np.float64)  # [8,128,18]
    cols = o.sum(axis=(0, 1))  # [18]
    s_recon = cols[0 : C_HV + 1].sum()  # combos 0..3 + all 8 halves
    s_enc2 = cols[C_ENC]
    s_cross = cols[C_CROSS]
    s_zsq = cols[C_ZSQ]
    g2 = o[0, :, C_G2].sum()  # replicated terms: core 0 only
    ra2 = o[0, :, C_A2].sum()

    pca_sq = s_enc2 - 2.0 * s_cross + s_zsq
    proj_sq = g2 - 2.0 * ra2 + float(I)
    loss = s_recon / B + 1.1 * pca_sq / B + 0.1 * proj_sq / (I * I)
    return np.asarray(loss, dtype=np.float32)
